# revision 42
# baseline (speedup 1.0000x reference)
"""Trainium2 Bass kernel for 16-head causal MHA (B=2, T=2048, D=1024, fp32 I/O).

Sharding: tensor-parallel over heads. Core c owns heads {2c, 2c+1}: it gets
Wq/Wk/Wv column slices [:, 128c:128c+128] and the Wo row slice
[128c:128c+128, :], computes its 2 heads' attention for both batch rows, and
produces a partial output [4096, 1024]; the host sums the 8 partials in f64.

Per-core device program (per batch), matmul inputs in bf16 (1 PE cycle/col +
fast weight load; fp32 would be 4 cycles/col), fp32 PSUM accumulation:
  - Q^T, K^T = W.T @ x^T  (x^T passed pre-transposed from host; weights
    stationary on PE, N=512 moving blocks)
  - V natural = x @ Wv    (x^T chunks stationary, Wv moving), with a ones
    column appended per head (from the all-ones mask columns)
  - attention in S^T layout: S^T[k,q] = K^T.T @ Q^T per 128-key x 512-query
    block (the two heads' K=64 contractions sit in PE row groups 0-63 /
    64-127 and run concurrently); exp on ScalarE with the 1/sqrt(dk) scale
    folded in; causality = skipping fully-masked blocks + multiplying
    diagonal blocks with a precomputed 0/1 staircase mask slice on VectorE.
  - ctx^T accumulation: lhsT = V block + ones column (M=65) so PSUM row 64
    accumulates the softmax denominator for free.
  - normalization is deferred out of the attention pipeline (PSUM frees
    immediately): unnormalized ctx^T and denominators are stashed to SBUF;
    then 1/den = exp(-ln(den)) on ScalarE (DVE reciprocal is ~3.3us/call),
    broadcast across partitions via a K=1 fp32 PE matmul with a ones
    stationary, and applied in-place on VectorE.
  - partial out = ctx^T.T @ Wo_c (single K=128 matmul per 128x512 block).

Infrastructure notes: the external walrus in this container allows only ONE
sync wait per instruction; Tile emits more, so a post-pass hoists extra waits
onto single-wait no-ops (and the TileContext closing drain is split into a
chain of single-wait drains).
"""

import numpy as np

import bass_rust
from bass_rust import ScopedClock
import concourse.bass as bass
import concourse.mybir as mybir
import concourse.tile as tile

F32 = mybir.dt.float32
BF16 = mybir.dt.bfloat16
# dtype for all PE-feeding tiles/inputs: bf16 streams 1 cycle/col on the PE
# (fp32r needs ~2) and enables fast weight load; inputs are cast on host.
F32R = BF16
B, T, D = 2, 2048, 1024
NCORES = 8
P = 128          # partitions / feature chunk
FC = D // P      # 8 feature chunks
QW = 512         # query block width (PSUM bank)
QN = T // QW     # 4 query blocks per batch
KC = T // P      # 16 key chunks per batch
NH = 2           # heads per core
DK = 64

# ---------------------------------------------------------------------------
# TileContext drain fix: the external walrus in this container allows only ONE
# sync wait per instruction, but Tile's closing drain packs one wait per active
# proc. Split it into a chain of single-wait drains (same semantics).
_PATCHED = False


def _patched_drain_and_barrier(self, tick_clock, wait_clock):
    nc = self.nc
    drain_inst = nc.sync.drain()
    wait_clock.add_sem_waits(
        drain_inst.ins, ScopedClock({None: tick_clock.global_clock})
    )
    si = drain_inst.ins.sync_info
    waits = list(si.on_wait) if si is not None else []
    if len(waits) > 1:
        si.on_wait = [waits[0]]
        drain_inst.ins.sync_info = si
        for w in waits[1:]:
            d2 = nc.sync.drain()
            si2 = d2.ins.sync_info
            if si2 is None:
                si2 = bass_rust.SyncInfo(on_wait=[w], on_update=[])
            else:
                si2.on_wait = [w]
            d2.ins.sync_info = si2
    nc.all_engine_barrier()
    assert self.sems is not None
    popped = nc._tile_sem_poison_stack.pop()
    assert popped is self._sem_poison
    nc.clear_and_free_semaphores(list(self.sems.allocated().values()))
    nc.all_engine_barrier()


def _apply_tile_patch():
    global _PATCHED
    if not _PATCHED:
        tile.TileContext._drain_and_barrier = _patched_drain_and_barrier
        _PATCHED = True


def _split_multi_waits(nc):
    """Post-pass: the external walrus accepts only 1 sync wait per
    instruction (2 for EventSemaphore). Tile emits more. Hoist extra waits
    onto same-engine no-ops inserted just before. For compute engines this
    is identical semantics (the engine blocks either way). For DMA triggers
    it turns queue-side waits into SP-side blocking, which is safe in this
    forward-dataflow single-block program (every wait's producer precedes
    the trigger in the scheduled stream); CoreSim re-validates no-deadlock."""
    for f in nc.m.functions:
        for bb in f.blocks:
            new = []
            for ins in bb.instructions:
                si = ins.sync_info
                if si is not None:
                    cap = 2 if isinstance(ins, mybir.InstEventSemaphore) else 1
                    waits = list(si.on_wait)
                    if len(waits) > cap:
                        for w in waits[:-cap]:
                            nop = mybir.InstNoOp(
                                name=nc.get_next_instruction_name(),
                                engine=ins.engine,
                                sync_info=bass_rust.SyncInfo(
                                    on_wait=[w], on_update=[]
                                ),
                                bass_nofuse=True,
                            )
                            nc.register_instruction(nop, overwrite=True)
                            new.append(nop)
                        si.on_wait = waits[-cap:]
                        ins.sync_info = si
                new.append(ins)
            bb.instructions = new


# ---------------------------------------------------------------------------
_PROGRAM = None


def build_program():
    global _PROGRAM
    if _PROGRAM is not None:
        return _PROGRAM
    _apply_tile_patch()
    Exp = mybir.ActivationFunctionType.Exp
    Log = mybir.ActivationFunctionType.Ln
    Copy = mybir.ActivationFunctionType.Copy

    # float32r tiles everywhere that feeds the PE: same 4-byte storage as
    # fp32, but matmuls stream at 1 cycle/row (vs 4 for fp32) when the
    # moving free dim is >= 256, at ~tf32 precision.
    nc = bass.Bass()
    xt_d = nc.declare_dram_parameter("xt", [D, B * T], F32R, isOutput=False)
    wq_d = nc.declare_dram_parameter("wq", [D, P], F32R, isOutput=False)
    wk_d = nc.declare_dram_parameter("wk", [D, P], F32R, isOutput=False)
    wv_d = nc.declare_dram_parameter("wv", [D, P], F32R, isOutput=False)
    wo_d = nc.declare_dram_parameter("wo", [P, D], F32R, isOutput=False)
    mask_d = nc.declare_dram_parameter("mask", [P, 896], F32R, isOutput=False)
    id_d = nc.declare_dram_parameter("ident", [P, P], F32R, isOutput=False)
    out_d = nc.declare_dram_parameter("out", [B * T, D], F32, isOutput=True)

    with tile.TileContext(nc) as tc:
        from contextlib import ExitStack

        ctx = ExitStack()
        with ctx:
            consts = ctx.enter_context(tc.tile_pool(name="consts", bufs=1))
            xt_pool = ctx.enter_context(tc.tile_pool(name="xt", bufs=8))
            qk_pool = ctx.enter_context(tc.tile_pool(name="qk", bufs=2))
            v_pool = ctx.enter_context(tc.tile_pool(name="v", bufs=2))
            exp_pool = ctx.enter_context(tc.tile_pool(name="exp", bufs=8))
            ctxt_pool = ctx.enter_context(tc.tile_pool(name="ctxt", bufs=2))
            ob_pool = ctx.enter_context(tc.tile_pool(name="ob", bufs=3))
            rec_pool = ctx.enter_context(tc.tile_pool(name="rec", bufs=1))

            ps_proj = ctx.enter_context(
                tc.tile_pool(name="ps_proj", bufs=2, space="PSUM")
            )
            ps_s = ctx.enter_context(tc.tile_pool(name="ps_s", bufs=2, space="PSUM"))
            ps_ctx = ctx.enter_context(
                tc.tile_pool(name="ps_ctx", bufs=1, space="PSUM")
            )

            # ---- constants ----
            wq_sb = consts.tile([P, FC, P], F32R, tag="wq")
            wk_sb = consts.tile([P, FC, P], F32R, tag="wk")
            wv_sb = consts.tile([P, FC, P], F32R, tag="wv")
            wo_sb = consts.tile([P, D], F32R, tag="wo")
            mask_sb = consts.tile([P, 896], F32R, tag="mask")
            ident_sb = consts.tile([P, P], F32R, tag="ident")
            ones_sb = consts.tile([65, DK], F32, tag="ones")
            nc.sync.dma_start(out=wq_sb, in_=wq_d.rearrange("(f p) c -> p f c", p=P))
            nc.sync.dma_start(out=wk_sb, in_=wk_d.rearrange("(f p) c -> p f c", p=P))
            nc.sync.dma_start(out=wv_sb, in_=wv_d.rearrange("(f p) c -> p f c", p=P))
            nc.sync.dma_start(out=wo_sb, in_=wo_d[:, :])
            nc.sync.dma_start(out=mask_sb, in_=mask_d[:, :])
            nc.sync.dma_start(out=ident_sb, in_=id_d[:, :])
            nc.vector.memset(ones_sb, 1.0)

            for b in range(B):
                # ---- load x^T chunks for this batch ----
                xts = []
                for fc in range(FC):
                    xt_t = xt_pool.tile([P, T], F32R, tag="xt")
                    nc.sync.dma_start(
                        out=xt_t,
                        in_=xt_d[fc * P : (fc + 1) * P, b * T : (b + 1) * T],
                    )
                    xts.append(xt_t)

                # ---- Q^T / K^T / V^T projections ----
                qt = qk_pool.tile([P, T], F32R, tag="qt")
                kt = qk_pool.tile([P, T], F32R, tag="kt")
                vt = qk_pool.tile([P, T], F32R, tag="vt")
                for w_sb, dst in ((wq_sb, qt), (wk_sb, kt), (wv_sb, vt)):
                    for rc in range(T // QW):
                        ps = ps_proj.tile([P, QW], F32, tag="proj")
                        for fc in range(FC):
                            nc.tensor.matmul(
                                ps,
                                lhsT=w_sb[:, fc, :],
                                rhs=xts[fc][:, rc * QW : (rc + 1) * QW],
                                start=(fc == 0),
                                stop=(fc == FC - 1),
                            )
                        nc.vector.tensor_copy(dst[:, rc * QW : (rc + 1) * QW], ps)

                # ---- V natural via PE transpose of V^T (bf16 transpose
                # streams 1 cycle/row; replaces 128 N=128 matmuls with 16
                # transposes per batch) ----
                v_sb = v_pool.tile([P, KC, 130], F32R, tag="v")
                for kc in range(KC):
                    ps = ps_proj.tile([P, P], F32R, tag="proj", name=f"vtr{kc}")
                    nc.tensor.transpose(ps, vt[:, kc * P : (kc + 1) * P], ident_sb)
                    nc.vector.tensor_copy(v_sb[:, kc, 0:DK], ps[:, 0:DK])
                    nc.vector.tensor_copy(v_sb[:, kc, 65 : 65 + DK], ps[:, DK:P])
                # ones columns for the denominator rows: mask cols 880..895
                # are all-ones; two strided copies fill all 16 chunks at once
                nc.vector.tensor_copy(
                    v_sb[:, :, 64:65],
                    mask_sb[:, 880:896].rearrange("p (c o) -> p c o", o=1),
                )
                nc.vector.tensor_copy(
                    v_sb[:, :, 129:130],
                    mask_sb[:, 880:896].rearrange("p (c o) -> p c o", o=1),
                )

                # ---- attention; normalization deferred out of the pipeline:
                # unnormalized ctx^T and denominators are stashed to SBUF so
                # the PSUM accumulators free up immediately ----
                ctxt = ctxt_pool.tile([P, T], F32R, tag="ctxt")
                den = rec_pool.tile([65, T], F32, tag="den")
                for qn in range(QN):
                    nkc = 4 * (qn + 1)  # live key chunks (causal)
                    ctx_ps = {
                        h: ps_ctx.tile([65, QW], F32, tag=f"ctx{h}", name=f"ctx{h}")
                        for h in range(NH)
                    }
                    for kc2 in range(0, nkc, 2):
                        es = {}
                        for h in range(NH):
                            # two key chunks land in one 2-bank psum tile so
                            # a single wide exp covers both (halves ScalarE
                            # per-op overhead and PE<->ACT handoffs)
                            s_ps = ps_s.tile([P, 2 * QW], F32, tag="s")
                            for half in range(2):
                                kc = kc2 + half
                                nc.tensor.matmul(
                                    s_ps[:, half * QW : (half + 1) * QW],
                                    lhsT=kt[
                                        h * DK : (h + 1) * DK, kc * P : (kc + 1) * P
                                    ],
                                    rhs=qt[
                                        h * DK : (h + 1) * DK,
                                        qn * QW : (qn + 1) * QW,
                                    ],
                                    start=True,
                                    stop=True,
                                )
                            e = exp_pool.tile([P, 2 * QW], F32R, tag="exp")
                            nc.scalar.activation(out=e, in_=s_ps, func=Exp, scale=0.125)
                            for half in range(2):
                                j = kc2 + half - 4 * qn
                                if j >= 0:  # diagonal block: causal staircase
                                    nc.vector.tensor_mul(
                                        e[:, half * QW : (half + 1) * QW],
                                        e[:, half * QW : (half + 1) * QW],
                                        mask_sb[:, 384 - 128 * j : 896 - 128 * j],
                                    )
                            es[h] = e
                        for h in range(NH):
                            for half in range(2):
                                kc = kc2 + half
                                nc.tensor.matmul(
                                    ctx_ps[h],
                                    lhsT=v_sb[:, kc, h * 65 : h * 65 + 65],
                                    rhs=es[h][:, half * QW : (half + 1) * QW],
                                    start=(kc == 0),
                                    stop=(kc == nkc - 1),
                                )
                    for h in range(NH):
                        nc.vector.tensor_copy(
                            ctxt[h * DK : (h + 1) * DK, qn * QW : (qn + 1) * QW],
                            ctx_ps[h][0:DK, :],
                        )
                        nc.vector.tensor_copy(
                            den[h * DK : h * DK + 1, qn * QW : (qn + 1) * QW],
                            ctx_ps[h][64:65, :],
                        )

                # ---- batched normalization: 1/den = exp(-ln(den)) on ScalarE
                # over all (h, qn) at once, then broadcast + in-place scale ----
                lnd = rec_pool.tile([65, T], F32, tag="rec")
                rcp = rec_pool.tile([65, T], F32, tag="rcp")
                for h in range(NH):
                    dp = h * DK
                    nc.scalar.activation(
                        out=lnd[dp : dp + 1, :], in_=den[dp : dp + 1, :], func=Log
                    )
                    nc.scalar.activation(
                        out=rcp[dp : dp + 1, :],
                        in_=lnd[dp : dp + 1, :],
                        func=Exp,
                        scale=-1.0,
                    )
                for qn in range(QN):
                    for h in range(NH):
                        bc_ps = ps_proj.tile(
                            [DK, QW], F32, tag="proj", name=f"bc{b}{h}{qn}"
                        )
                        nc.tensor.matmul(
                            bc_ps,
                            lhsT=ones_sb[h * DK : h * DK + 1, :],
                            rhs=rcp[h * DK : h * DK + 1, qn * QW : (qn + 1) * QW],
                            start=True,
                            stop=True,
                        )
                        nc.vector.tensor_mul(
                            ctxt[h * DK : (h + 1) * DK, qn * QW : (qn + 1) * QW],
                            ctxt[h * DK : (h + 1) * DK, qn * QW : (qn + 1) * QW],
                            bc_ps,
                        )

                # ---- output projection (partial over this core's heads) ----
                for rc in range(T // P):
                    for c2 in range(D // QW):
                        ps = ps_proj.tile([P, QW], F32, tag="proj")
                        nc.tensor.matmul(
                            ps,
                            lhsT=ctxt[:, rc * P : (rc + 1) * P],
                            rhs=wo_sb[:, c2 * QW : (c2 + 1) * QW],
                            start=True,
                            stop=True,
                        )
                        ob = ob_pool.tile([P, QW], F32, tag="ob")
                        nc.vector.tensor_copy(ob, ps)
                        nc.sync.dma_start(
                            out=out_d[
                                b * T + rc * P : b * T + (rc + 1) * P,
                                c2 * QW : (c2 + 1) * QW,
                            ],
                            in_=ob,
                        )

    _split_multi_waits(nc)
    _PROGRAM = nc
    return nc


def _make_mask():
    # mask[i, u] = 1.0 if u >= i + 384 else 0.0   (shape [128, 896])
    i = np.arange(P)[:, None]
    u = np.arange(896)[None, :]
    return (u >= i + 384).astype(np.float32)


def make_in_maps(x, Wq, Wk, Wv, Wo):
    import ml_dtypes

    nd = ml_dtypes.bfloat16 if F32R == BF16 else np.float32
    x = np.asarray(x, dtype=np.float32)
    xt = np.ascontiguousarray(x.reshape(B * T, D).T).astype(nd)  # [1024, 4096]
    mask = _make_mask().astype(nd)
    ident = np.eye(P, dtype=np.float32).astype(nd)
    Wq, Wk, Wv, Wo = (np.asarray(w, dtype=np.float32) for w in (Wq, Wk, Wv, Wo))
    in_maps = []
    for c in range(NCORES):
        cols = slice(c * P, (c + 1) * P)
        in_maps.append(
            {
                "xt": xt,
                "wq": np.ascontiguousarray(Wq[:, cols]).astype(nd),
                "wk": np.ascontiguousarray(Wk[:, cols]).astype(nd),
                "wv": np.ascontiguousarray(Wv[:, cols]).astype(nd),
                "wo": np.ascontiguousarray(Wo[cols, :]).astype(nd),
                "mask": mask,
                "ident": ident,
            }
        )
    return in_maps


def kernel(x, Wq, Wk, Wv, Wo):
    from concourse.bass_utils import run_bass_kernel_spmd

    nc = build_program()
    in_maps = make_in_maps(x, Wq, Wk, Wv, Wo)
    res = run_bass_kernel_spmd(nc, in_maps, core_ids=list(range(NCORES)))
    acc = np.zeros((B * T, D), dtype=np.float64)
    for c in range(NCORES):
        acc += res.results[c]["out"]
    return acc.astype(np.float32).reshape(B, T, D)


if __name__ == "__main__":
    rng = np.random.default_rng(0)
    s = 1.0 / np.sqrt(D)
    ins = {
        "x": rng.standard_normal((B, T, D)).astype(np.float32),
        "Wq": (rng.standard_normal((D, D)) * s).astype(np.float32),
        "Wk": (rng.standard_normal((D, D)) * s).astype(np.float32),
        "Wv": (rng.standard_normal((D, D)) * s).astype(np.float32),
        "Wo": (rng.standard_normal((D, D)) * (1.0 / np.sqrt(D))).astype(np.float32),
    }
    out = kernel(**ins)
    print("out", out.shape, out.dtype, float(np.abs(out).max()))


# revision 44
# speedup vs baseline: 1.0289x; 1.0289x over previous
"""Trainium2 Bass kernel for 16-head causal MHA (B=2, T=2048, D=1024, fp32 I/O).

Sharding: tensor-parallel over heads. Core c owns heads {2c, 2c+1}: it gets
Wq/Wk/Wv column slices [:, 128c:128c+128] and the Wo row slice
[128c:128c+128, :], computes its 2 heads' attention for both batch rows, and
produces a partial output [4096, 1024]; the host sums the 8 partials in f64.

Per-core device program (per batch), matmul inputs in bf16 (1 PE cycle/col +
fast weight load; fp32 would be 4 cycles/col), fp32 PSUM accumulation:
  - Q^T, K^T = W.T @ x^T  (x^T passed pre-transposed from host; weights
    stationary on PE, N=512 moving blocks)
  - V natural = x @ Wv    (x^T chunks stationary, Wv moving), with a ones
    column appended per head (from the all-ones mask columns)
  - attention in S^T layout: S^T[k,q] = K^T.T @ Q^T per 128-key x 512-query
    block (the two heads' K=64 contractions sit in PE row groups 0-63 /
    64-127 and run concurrently); exp on ScalarE with the 1/sqrt(dk) scale
    folded in; causality = skipping fully-masked blocks + multiplying
    diagonal blocks with a precomputed 0/1 staircase mask slice on VectorE.
  - ctx^T accumulation: lhsT = V block + ones column (M=65) so PSUM row 64
    accumulates the softmax denominator for free.
  - normalization is deferred out of the attention pipeline (PSUM frees
    immediately): unnormalized ctx^T and denominators are stashed to SBUF;
    then 1/den = exp(-ln(den)) on ScalarE (DVE reciprocal is ~3.3us/call),
    broadcast across partitions via a K=1 fp32 PE matmul with a ones
    stationary, and applied in-place on VectorE.
  - partial out = ctx^T.T @ Wo_c (single K=128 matmul per 128x512 block).

Infrastructure notes: the external walrus in this container allows only ONE
sync wait per instruction; Tile emits more, so a post-pass hoists extra waits
onto single-wait no-ops (and the TileContext closing drain is split into a
chain of single-wait drains).
"""

import numpy as np

import bass_rust
from bass_rust import ScopedClock
import concourse.bass as bass
import concourse.mybir as mybir
import concourse.tile as tile

F32 = mybir.dt.float32
BF16 = mybir.dt.bfloat16
# dtype for all PE-feeding tiles/inputs: bf16 streams 1 cycle/col on the PE
# (fp32r needs ~2) and enables fast weight load; inputs are cast on host.
F32R = BF16
B, T, D = 2, 2048, 1024
NCORES = 8
P = 128          # partitions / feature chunk
FC = D // P      # 8 feature chunks
QW = 512         # query block width (PSUM bank)
QN = T // QW     # 4 query blocks per batch
KC = T // P      # 16 key chunks per batch
NH = 2           # heads per core
DK = 64

# ---------------------------------------------------------------------------
# TileContext drain fix: the external walrus in this container allows only ONE
# sync wait per instruction, but Tile's closing drain packs one wait per active
# proc. Split it into a chain of single-wait drains (same semantics).
_PATCHED = False


def _patched_drain_and_barrier(self, tick_clock, wait_clock):
    nc = self.nc
    drain_inst = nc.sync.drain()
    wait_clock.add_sem_waits(
        drain_inst.ins, ScopedClock({None: tick_clock.global_clock})
    )
    si = drain_inst.ins.sync_info
    waits = list(si.on_wait) if si is not None else []
    if len(waits) > 1:
        si.on_wait = [waits[0]]
        drain_inst.ins.sync_info = si
        for w in waits[1:]:
            d2 = nc.sync.drain()
            si2 = d2.ins.sync_info
            if si2 is None:
                si2 = bass_rust.SyncInfo(on_wait=[w], on_update=[])
            else:
                si2.on_wait = [w]
            d2.ins.sync_info = si2
    nc.all_engine_barrier()
    assert self.sems is not None
    popped = nc._tile_sem_poison_stack.pop()
    assert popped is self._sem_poison
    nc.clear_and_free_semaphores(list(self.sems.allocated().values()))
    nc.all_engine_barrier()


def _apply_tile_patch():
    global _PATCHED
    if not _PATCHED:
        tile.TileContext._drain_and_barrier = _patched_drain_and_barrier
        _PATCHED = True


def _split_multi_waits(nc):
    """Post-pass: the external walrus accepts only 1 sync wait per
    instruction (2 for EventSemaphore). Tile emits more. Hoist extra waits
    onto same-engine no-ops inserted just before. For compute engines this
    is identical semantics (the engine blocks either way). For DMA triggers
    it turns queue-side waits into SP-side blocking, which is safe in this
    forward-dataflow single-block program (every wait's producer precedes
    the trigger in the scheduled stream); CoreSim re-validates no-deadlock."""
    for f in nc.m.functions:
        for bb in f.blocks:
            new = []
            for ins in bb.instructions:
                si = ins.sync_info
                if si is not None:
                    cap = 2 if isinstance(ins, mybir.InstEventSemaphore) else 1
                    waits = list(si.on_wait)
                    if len(waits) > cap:
                        for w in waits[:-cap]:
                            nop = mybir.InstNoOp(
                                name=nc.get_next_instruction_name(),
                                engine=ins.engine,
                                sync_info=bass_rust.SyncInfo(
                                    on_wait=[w], on_update=[]
                                ),
                                bass_nofuse=True,
                            )
                            nc.register_instruction(nop, overwrite=True)
                            new.append(nop)
                        si.on_wait = waits[-cap:]
                        ins.sync_info = si
                new.append(ins)
            bb.instructions = new


# ---------------------------------------------------------------------------
_PROGRAM = None


def build_program():
    global _PROGRAM
    if _PROGRAM is not None:
        return _PROGRAM
    _apply_tile_patch()
    Exp = mybir.ActivationFunctionType.Exp
    Log = mybir.ActivationFunctionType.Ln
    Copy = mybir.ActivationFunctionType.Copy

    # float32r tiles everywhere that feeds the PE: same 4-byte storage as
    # fp32, but matmuls stream at 1 cycle/row (vs 4 for fp32) when the
    # moving free dim is >= 256, at ~tf32 precision.
    nc = bass.Bass()
    xt_d = nc.declare_dram_parameter("xt", [D, B * T], F32R, isOutput=False)
    wq_d = nc.declare_dram_parameter("wq", [D, P], F32R, isOutput=False)
    wk_d = nc.declare_dram_parameter("wk", [D, P], F32R, isOutput=False)
    wv_d = nc.declare_dram_parameter("wv", [D, P], F32R, isOutput=False)
    wo_d = nc.declare_dram_parameter("wo", [P, D], F32R, isOutput=False)
    mask_d = nc.declare_dram_parameter("mask", [P, 896], F32R, isOutput=False)
    id_d = nc.declare_dram_parameter("ident", [P, P], F32R, isOutput=False)
    out_d = nc.declare_dram_parameter("out", [B * T, D], F32, isOutput=True)

    with tile.TileContext(nc) as tc:
        from contextlib import ExitStack

        ctx = ExitStack()
        with ctx:
            consts = ctx.enter_context(tc.tile_pool(name="consts", bufs=1))
            xt_pool = ctx.enter_context(tc.tile_pool(name="xt", bufs=8))
            qk_pool = ctx.enter_context(tc.tile_pool(name="qk", bufs=2))
            v_pool = ctx.enter_context(tc.tile_pool(name="v", bufs=2))
            exp_pool = ctx.enter_context(tc.tile_pool(name="exp", bufs=8))
            ctxt_pool = ctx.enter_context(tc.tile_pool(name="ctxt", bufs=2))
            ob_pool = ctx.enter_context(tc.tile_pool(name="ob", bufs=3))
            rec_pool = ctx.enter_context(tc.tile_pool(name="rec", bufs=1))

            ps_proj = ctx.enter_context(
                tc.tile_pool(name="ps_proj", bufs=2, space="PSUM")
            )
            ps_s = ctx.enter_context(tc.tile_pool(name="ps_s", bufs=2, space="PSUM"))
            ps_ctx = ctx.enter_context(
                tc.tile_pool(name="ps_ctx", bufs=1, space="PSUM")
            )

            # ---- constants ----
            wq_sb = consts.tile([P, FC, P], F32R, tag="wq")
            wk_sb = consts.tile([P, FC, P], F32R, tag="wk")
            wv_sb = consts.tile([P, FC, P], F32R, tag="wv")
            wo_sb = consts.tile([P, D], F32R, tag="wo")
            mask_sb = consts.tile([P, 896], F32R, tag="mask")
            ident_sb = consts.tile([P, P], F32R, tag="ident")
            ones_sb = consts.tile([65, DK], F32, tag="ones")
            nc.sync.dma_start(out=wq_sb, in_=wq_d.rearrange("(f p) c -> p f c", p=P))
            nc.sync.dma_start(out=wk_sb, in_=wk_d.rearrange("(f p) c -> p f c", p=P))
            nc.sync.dma_start(out=wv_sb, in_=wv_d.rearrange("(f p) c -> p f c", p=P))
            nc.sync.dma_start(out=wo_sb, in_=wo_d[:, :])
            nc.sync.dma_start(out=mask_sb, in_=mask_d[:, :])
            nc.sync.dma_start(out=ident_sb, in_=id_d[:, :])
            nc.vector.memset(ones_sb, 1.0)

            for b in range(B):
                # ---- load x^T chunks for this batch ----
                xts = []
                for fc in range(FC):
                    xt_t = xt_pool.tile([P, T], F32R, tag="xt")
                    nc.sync.dma_start(
                        out=xt_t,
                        in_=xt_d[fc * P : (fc + 1) * P, b * T : (b + 1) * T],
                    )
                    xts.append(xt_t)

                # ---- Q^T / K^T projections ----
                qt = qk_pool.tile([P, T], F32R, tag="qt")
                kt = qk_pool.tile([P, T], F32R, tag="kt")
                for w_sb, dst in ((wq_sb, qt), (wk_sb, kt)):
                    for rc in range(T // QW):
                        ps = ps_proj.tile([P, QW], F32, tag="proj")
                        for fc in range(FC):
                            nc.tensor.matmul(
                                ps,
                                lhsT=w_sb[:, fc, :],
                                rhs=xts[fc][:, rc * QW : (rc + 1) * QW],
                                start=(fc == 0),
                                stop=(fc == FC - 1),
                            )
                        nc.vector.tensor_copy(dst[:, rc * QW : (rc + 1) * QW], ps)

                # ---- V natural (direct; with ones columns for denom) ----
                v_sb = v_pool.tile([P, KC, 130], F32R, tag="v")
                for kc in range(KC):
                    ps = ps_proj.tile([P, P], F32, tag="proj", name=f"vps{kc}")
                    for fc in range(FC):
                        nc.tensor.matmul(
                            ps,
                            lhsT=xts[fc][:, kc * P : (kc + 1) * P],
                            rhs=wv_sb[:, fc, :],
                            start=(fc == 0),
                            stop=(fc == FC - 1),
                        )
                    nc.vector.tensor_copy(v_sb[:, kc, 0:DK], ps[:, 0:DK])
                    nc.vector.tensor_copy(v_sb[:, kc, 65 : 65 + DK], ps[:, DK:P])
                # ones columns for the denominator rows: mask cols 880..895
                # are all-ones; two strided copies fill all 16 chunks at once
                nc.vector.tensor_copy(
                    v_sb[:, :, 64:65],
                    mask_sb[:, 880:896].rearrange("p (c o) -> p c o", o=1),
                )
                nc.vector.tensor_copy(
                    v_sb[:, :, 129:130],
                    mask_sb[:, 880:896].rearrange("p (c o) -> p c o", o=1),
                )

                # ---- attention; normalization deferred out of the pipeline:
                # unnormalized ctx^T and denominators are stashed to SBUF so
                # the PSUM accumulators free up immediately ----
                ctxt = ctxt_pool.tile([P, T], F32R, tag="ctxt")
                den = rec_pool.tile([65, T], F32, tag="den")
                for qn in range(QN):
                    nkc = 4 * (qn + 1)  # live key chunks (causal)
                    ctx_ps = {
                        h: ps_ctx.tile([65, QW], F32, tag=f"ctx{h}", name=f"ctx{h}")
                        for h in range(NH)
                    }
                    for kc2 in range(0, nkc, 2):
                        es = {}
                        for h in range(NH):
                            # two key chunks land in one 2-bank psum tile so
                            # a single wide exp covers both (halves ScalarE
                            # per-op overhead and PE<->ACT handoffs)
                            s_ps = ps_s.tile([P, 2 * QW], F32, tag="s")
                            for half in range(2):
                                kc = kc2 + half
                                nc.tensor.matmul(
                                    s_ps[:, half * QW : (half + 1) * QW],
                                    lhsT=kt[
                                        h * DK : (h + 1) * DK, kc * P : (kc + 1) * P
                                    ],
                                    rhs=qt[
                                        h * DK : (h + 1) * DK,
                                        qn * QW : (qn + 1) * QW,
                                    ],
                                    start=True,
                                    stop=True,
                                )
                            e = exp_pool.tile([P, 2 * QW], F32R, tag="exp")
                            nc.scalar.activation(out=e, in_=s_ps, func=Exp, scale=0.125)
                            for half in range(2):
                                j = kc2 + half - 4 * qn
                                if j >= 0:  # diagonal block: causal staircase
                                    nc.vector.tensor_mul(
                                        e[:, half * QW : (half + 1) * QW],
                                        e[:, half * QW : (half + 1) * QW],
                                        mask_sb[:, 384 - 128 * j : 896 - 128 * j],
                                    )
                            es[h] = e
                        for h in range(NH):
                            for half in range(2):
                                kc = kc2 + half
                                nc.tensor.matmul(
                                    ctx_ps[h],
                                    lhsT=v_sb[:, kc, h * 65 : h * 65 + 65],
                                    rhs=es[h][:, half * QW : (half + 1) * QW],
                                    start=(kc == 0),
                                    stop=(kc == nkc - 1),
                                )
                    for h in range(NH):
                        nc.vector.tensor_copy(
                            ctxt[h * DK : (h + 1) * DK, qn * QW : (qn + 1) * QW],
                            ctx_ps[h][0:DK, :],
                        )
                        nc.scalar.activation(
                            out=den[h * DK : h * DK + 1, qn * QW : (qn + 1) * QW],
                            in_=ctx_ps[h][64:65, :],
                            func=Copy,
                        )

                # ---- batched normalization: 1/den = exp(-ln(den)) on ScalarE
                # over all (h, qn) at once, then broadcast + in-place scale ----
                lnd = rec_pool.tile([65, T], F32, tag="rec")
                rcp = rec_pool.tile([65, T], F32, tag="rcp")
                for h in range(NH):
                    dp = h * DK
                    nc.scalar.activation(
                        out=lnd[dp : dp + 1, :], in_=den[dp : dp + 1, :], func=Log
                    )
                    nc.scalar.activation(
                        out=rcp[dp : dp + 1, :],
                        in_=lnd[dp : dp + 1, :],
                        func=Exp,
                        scale=-1.0,
                    )
                for qn in range(QN):
                    for h in range(NH):
                        bc_ps = ps_proj.tile(
                            [DK, QW], F32, tag="proj", name=f"bc{b}{h}{qn}"
                        )
                        nc.tensor.matmul(
                            bc_ps,
                            lhsT=ones_sb[h * DK : h * DK + 1, :],
                            rhs=rcp[h * DK : h * DK + 1, qn * QW : (qn + 1) * QW],
                            start=True,
                            stop=True,
                        )
                        nc.vector.tensor_mul(
                            ctxt[h * DK : (h + 1) * DK, qn * QW : (qn + 1) * QW],
                            ctxt[h * DK : (h + 1) * DK, qn * QW : (qn + 1) * QW],
                            bc_ps,
                        )

                # ---- output projection (partial over this core's heads) ----
                for rc in range(T // P):
                    for c2 in range(D // QW):
                        ps = ps_proj.tile([P, QW], F32, tag="proj")
                        nc.tensor.matmul(
                            ps,
                            lhsT=ctxt[:, rc * P : (rc + 1) * P],
                            rhs=wo_sb[:, c2 * QW : (c2 + 1) * QW],
                            start=True,
                            stop=True,
                        )
                        ob = ob_pool.tile([P, QW], F32, tag="ob")
                        if (rc + c2) % 2 == 0:
                            nc.vector.tensor_copy(ob, ps)
                        else:
                            nc.scalar.activation(out=ob, in_=ps, func=Copy)
                        nc.sync.dma_start(
                            out=out_d[
                                b * T + rc * P : b * T + (rc + 1) * P,
                                c2 * QW : (c2 + 1) * QW,
                            ],
                            in_=ob,
                        )

    _split_multi_waits(nc)
    _PROGRAM = nc
    return nc


def _make_mask():
    # mask[i, u] = 1.0 if u >= i + 384 else 0.0   (shape [128, 896])
    i = np.arange(P)[:, None]
    u = np.arange(896)[None, :]
    return (u >= i + 384).astype(np.float32)


def make_in_maps(x, Wq, Wk, Wv, Wo):
    import ml_dtypes

    nd = ml_dtypes.bfloat16 if F32R == BF16 else np.float32
    x = np.asarray(x, dtype=np.float32)
    xt = np.ascontiguousarray(x.reshape(B * T, D).T).astype(nd)  # [1024, 4096]
    mask = _make_mask().astype(nd)
    ident = np.eye(P, dtype=np.float32).astype(nd)
    Wq, Wk, Wv, Wo = (np.asarray(w, dtype=np.float32) for w in (Wq, Wk, Wv, Wo))
    in_maps = []
    for c in range(NCORES):
        cols = slice(c * P, (c + 1) * P)
        in_maps.append(
            {
                "xt": xt,
                "wq": np.ascontiguousarray(Wq[:, cols]).astype(nd),
                "wk": np.ascontiguousarray(Wk[:, cols]).astype(nd),
                "wv": np.ascontiguousarray(Wv[:, cols]).astype(nd),
                "wo": np.ascontiguousarray(Wo[cols, :]).astype(nd),
                "mask": mask,
                "ident": ident,
            }
        )
    return in_maps


def kernel(x, Wq, Wk, Wv, Wo):
    from concourse.bass_utils import run_bass_kernel_spmd

    nc = build_program()
    in_maps = make_in_maps(x, Wq, Wk, Wv, Wo)
    res = run_bass_kernel_spmd(nc, in_maps, core_ids=list(range(NCORES)))
    acc = np.zeros((B * T, D), dtype=np.float64)
    for c in range(NCORES):
        acc += res.results[c]["out"]
    return acc.astype(np.float32).reshape(B, T, D)


if __name__ == "__main__":
    rng = np.random.default_rng(0)
    s = 1.0 / np.sqrt(D)
    ins = {
        "x": rng.standard_normal((B, T, D)).astype(np.float32),
        "Wq": (rng.standard_normal((D, D)) * s).astype(np.float32),
        "Wk": (rng.standard_normal((D, D)) * s).astype(np.float32),
        "Wv": (rng.standard_normal((D, D)) * s).astype(np.float32),
        "Wo": (rng.standard_normal((D, D)) * (1.0 / np.sqrt(D))).astype(np.float32),
    }
    out = kernel(**ins)
    print("out", out.shape, out.dtype, float(np.abs(out).max()))


# revision 45
# speedup vs baseline: 1.0428x; 1.0135x over previous
"""Trainium2 Bass kernel for 16-head causal MHA (B=2, T=2048, D=1024, fp32 I/O).

Sharding: tensor-parallel over heads. Core c owns heads {2c, 2c+1}: it gets
Wq/Wk/Wv column slices [:, 128c:128c+128] and the Wo row slice
[128c:128c+128, :], computes its 2 heads' attention for both batch rows, and
produces a partial output [4096, 1024]; the host sums the 8 partials in f64.

Per-core device program (per batch), matmul inputs in bf16 (1 PE cycle/col +
fast weight load; fp32 would be 4 cycles/col), fp32 PSUM accumulation:
  - Q^T, K^T = W.T @ x^T  (x^T passed pre-transposed from host; weights
    stationary on PE, N=512 moving blocks)
  - V natural = x @ Wv    (x^T chunks stationary, Wv moving), with a ones
    column appended per head (from the all-ones mask columns)
  - attention in S^T layout: S^T[k,q] = K^T.T @ Q^T per 128-key x 512-query
    block (the two heads' K=64 contractions sit in PE row groups 0-63 /
    64-127 and run concurrently); exp on ScalarE with the 1/sqrt(dk) scale
    folded in; causality = skipping fully-masked blocks + multiplying
    diagonal blocks with a precomputed 0/1 staircase mask slice on VectorE.
  - ctx^T accumulation: lhsT = V block + ones column (M=65) so PSUM row 64
    accumulates the softmax denominator for free.
  - normalization is deferred out of the attention pipeline (PSUM frees
    immediately): unnormalized ctx^T and denominators are stashed to SBUF;
    then 1/den = exp(-ln(den)) on ScalarE (DVE reciprocal is ~3.3us/call),
    broadcast across partitions via a K=1 fp32 PE matmul with a ones
    stationary, and applied in-place on VectorE.
  - partial out = ctx^T.T @ Wo_c (single K=128 matmul per 128x512 block).

Infrastructure notes: the external walrus in this container allows only ONE
sync wait per instruction; Tile emits more, so a post-pass hoists extra waits
onto single-wait no-ops (and the TileContext closing drain is split into a
chain of single-wait drains).
"""

import numpy as np

import bass_rust
from bass_rust import ScopedClock
import concourse.bass as bass
import concourse.mybir as mybir
import concourse.tile as tile

F32 = mybir.dt.float32
BF16 = mybir.dt.bfloat16
# dtype for all PE-feeding tiles/inputs: bf16 streams 1 cycle/col on the PE
# (fp32r needs ~2) and enables fast weight load; inputs are cast on host.
F32R = BF16
B, T, D = 2, 2048, 1024
NCORES = 8
P = 128          # partitions / feature chunk
FC = D // P      # 8 feature chunks
QW = 512         # query block width (PSUM bank)
QN = T // QW     # 4 query blocks per batch
KC = T // P      # 16 key chunks per batch
NH = 2           # heads per core
DK = 64

# ---------------------------------------------------------------------------
# TileContext drain fix: the external walrus in this container allows only ONE
# sync wait per instruction, but Tile's closing drain packs one wait per active
# proc. Split it into a chain of single-wait drains (same semantics).
_PATCHED = False


def _patched_drain_and_barrier(self, tick_clock, wait_clock):
    nc = self.nc
    drain_inst = nc.sync.drain()
    wait_clock.add_sem_waits(
        drain_inst.ins, ScopedClock({None: tick_clock.global_clock})
    )
    si = drain_inst.ins.sync_info
    waits = list(si.on_wait) if si is not None else []
    if len(waits) > 1:
        si.on_wait = [waits[0]]
        drain_inst.ins.sync_info = si
        for w in waits[1:]:
            d2 = nc.sync.drain()
            si2 = d2.ins.sync_info
            if si2 is None:
                si2 = bass_rust.SyncInfo(on_wait=[w], on_update=[])
            else:
                si2.on_wait = [w]
            d2.ins.sync_info = si2
    nc.all_engine_barrier()
    assert self.sems is not None
    popped = nc._tile_sem_poison_stack.pop()
    assert popped is self._sem_poison
    nc.clear_and_free_semaphores(list(self.sems.allocated().values()))
    nc.all_engine_barrier()


def _apply_tile_patch():
    global _PATCHED
    if not _PATCHED:
        tile.TileContext._drain_and_barrier = _patched_drain_and_barrier
        _PATCHED = True


def _split_multi_waits(nc):
    """Post-pass: the external walrus accepts only 1 sync wait per
    instruction (2 for EventSemaphore). Tile emits more. Hoist extra waits
    onto same-engine no-ops inserted just before. For compute engines this
    is identical semantics (the engine blocks either way). For DMA triggers
    it turns queue-side waits into SP-side blocking, which is safe in this
    forward-dataflow single-block program (every wait's producer precedes
    the trigger in the scheduled stream); CoreSim re-validates no-deadlock."""
    for f in nc.m.functions:
        for bb in f.blocks:
            new = []
            for ins in bb.instructions:
                si = ins.sync_info
                if si is not None:
                    cap = 2 if isinstance(ins, mybir.InstEventSemaphore) else 1
                    waits = list(si.on_wait)
                    if len(waits) > cap:
                        for w in waits[:-cap]:
                            nop = mybir.InstNoOp(
                                name=nc.get_next_instruction_name(),
                                engine=ins.engine,
                                sync_info=bass_rust.SyncInfo(
                                    on_wait=[w], on_update=[]
                                ),
                                bass_nofuse=True,
                            )
                            nc.register_instruction(nop, overwrite=True)
                            new.append(nop)
                        si.on_wait = waits[-cap:]
                        ins.sync_info = si
                new.append(ins)
            bb.instructions = new


# ---------------------------------------------------------------------------
_PROGRAM = None


def build_program():
    global _PROGRAM
    if _PROGRAM is not None:
        return _PROGRAM
    _apply_tile_patch()
    Exp = mybir.ActivationFunctionType.Exp
    Log = mybir.ActivationFunctionType.Ln
    Copy = mybir.ActivationFunctionType.Copy

    # float32r tiles everywhere that feeds the PE: same 4-byte storage as
    # fp32, but matmuls stream at 1 cycle/row (vs 4 for fp32) when the
    # moving free dim is >= 256, at ~tf32 precision.
    nc = bass.Bass()
    xt_d = nc.declare_dram_parameter("xt", [D, B * T], F32R, isOutput=False)
    wq_d = nc.declare_dram_parameter("wq", [D, P], F32R, isOutput=False)
    wk_d = nc.declare_dram_parameter("wk", [D, P], F32R, isOutput=False)
    wv_d = nc.declare_dram_parameter("wv", [D, P], F32R, isOutput=False)
    wo_d = nc.declare_dram_parameter("wo", [P, D], F32R, isOutput=False)
    mask_d = nc.declare_dram_parameter("mask", [P, 896], F32R, isOutput=False)
    id_d = nc.declare_dram_parameter("ident", [P, P], F32R, isOutput=False)
    out_d = nc.declare_dram_parameter("out", [B * T, D], F32, isOutput=True)

    with tile.TileContext(nc) as tc:
        from contextlib import ExitStack

        ctx = ExitStack()
        with ctx:
            consts = ctx.enter_context(tc.tile_pool(name="consts", bufs=1))
            xt_pool = ctx.enter_context(tc.tile_pool(name="xt", bufs=8))
            qk_pool = ctx.enter_context(tc.tile_pool(name="qk", bufs=2))
            v_pool = ctx.enter_context(tc.tile_pool(name="v", bufs=2))
            exp_pool = ctx.enter_context(tc.tile_pool(name="exp", bufs=8))
            ctxt_pool = ctx.enter_context(tc.tile_pool(name="ctxt", bufs=2))
            ob_pool = ctx.enter_context(tc.tile_pool(name="ob", bufs=3))
            rec_pool = ctx.enter_context(tc.tile_pool(name="rec", bufs=1))

            ps_proj = ctx.enter_context(
                tc.tile_pool(name="ps_proj", bufs=2, space="PSUM")
            )
            ps_s = ctx.enter_context(tc.tile_pool(name="ps_s", bufs=2, space="PSUM"))
            ps_ctx = ctx.enter_context(
                tc.tile_pool(name="ps_ctx", bufs=1, space="PSUM")
            )

            # ---- constants ----
            wq_sb = consts.tile([P, FC, P], F32R, tag="wq")
            wk_sb = consts.tile([P, FC, P], F32R, tag="wk")
            wv_sb = consts.tile([P, FC, P], F32R, tag="wv")
            wo_sb = consts.tile([P, D], F32R, tag="wo")
            mask_sb = consts.tile([P, 896], F32R, tag="mask")
            ident_sb = consts.tile([P, P], F32R, tag="ident")
            ones_sb = consts.tile([65, DK], F32, tag="ones")
            nc.sync.dma_start(out=wq_sb, in_=wq_d.rearrange("(f p) c -> p f c", p=P))
            nc.sync.dma_start(out=wk_sb, in_=wk_d.rearrange("(f p) c -> p f c", p=P))
            nc.sync.dma_start(out=wv_sb, in_=wv_d.rearrange("(f p) c -> p f c", p=P))
            nc.sync.dma_start(out=wo_sb, in_=wo_d[:, :])
            nc.sync.dma_start(out=mask_sb, in_=mask_d[:, :])
            nc.sync.dma_start(out=ident_sb, in_=id_d[:, :])
            nc.vector.memset(ones_sb, 1.0)

            for b in range(B):
                # ---- load x^T chunks for this batch ----
                xts = []
                for fc in range(FC):
                    xt_t = xt_pool.tile([P, T], F32R, tag="xt")
                    nc.sync.dma_start(
                        out=xt_t,
                        in_=xt_d[fc * P : (fc + 1) * P, b * T : (b + 1) * T],
                    )
                    xts.append(xt_t)

                # ---- Q^T / K^T projections ----
                qt = qk_pool.tile([P, T], F32R, tag="qt")
                kt = qk_pool.tile([P, T], F32R, tag="kt")
                for w_sb, dst in ((wq_sb, qt), (wk_sb, kt)):
                    for rc in range(T // QW):
                        ps = ps_proj.tile([P, QW], F32, tag="proj")
                        for fc in range(FC):
                            nc.tensor.matmul(
                                ps,
                                lhsT=w_sb[:, fc, :],
                                rhs=xts[fc][:, rc * QW : (rc + 1) * QW],
                                start=(fc == 0),
                                stop=(fc == FC - 1),
                            )
                        nc.vector.tensor_copy(dst[:, rc * QW : (rc + 1) * QW], ps)

                # ---- V natural (direct; with ones columns for denom) ----
                v_sb = v_pool.tile([P, KC, 130], F32R, tag="v")
                for kc in range(KC):
                    ps = ps_proj.tile([P, P], F32, tag="proj", name=f"vps{kc}")
                    for fc in range(FC):
                        nc.tensor.matmul(
                            ps,
                            lhsT=xts[fc][:, kc * P : (kc + 1) * P],
                            rhs=wv_sb[:, fc, :],
                            start=(fc == 0),
                            stop=(fc == FC - 1),
                        )
                    nc.vector.tensor_copy(v_sb[:, kc, 0:DK], ps[:, 0:DK])
                    nc.vector.tensor_copy(v_sb[:, kc, 65 : 65 + DK], ps[:, DK:P])
                # ones columns for the denominator rows: mask cols 880..895
                # are all-ones; two strided copies fill all 16 chunks at once
                nc.vector.tensor_copy(
                    v_sb[:, :, 64:65],
                    mask_sb[:, 880:896].rearrange("p (c o) -> p c o", o=1),
                )
                nc.vector.tensor_copy(
                    v_sb[:, :, 129:130],
                    mask_sb[:, 880:896].rearrange("p (c o) -> p c o", o=1),
                )

                # ---- attention; normalization deferred out of the pipeline:
                # unnormalized ctx^T and denominators are stashed to SBUF so
                # the PSUM accumulators free up immediately ----
                ctxt = ctxt_pool.tile([P, T], F32R, tag="ctxt")
                den = rec_pool.tile([65, T], F32, tag="den")
                for qn in range(QN):
                    nkc = 4 * (qn + 1)  # live key chunks (causal)
                    ctx_ps = {
                        h: ps_ctx.tile([65, QW], F32, tag=f"ctx{h}", name=f"ctx{h}")
                        for h in range(NH)
                    }
                    for kc2 in range(0, nkc, 2):
                        es = {}
                        for h in range(NH):
                            # two key chunks land in one 2-bank psum tile so
                            # a single wide exp covers both (halves ScalarE
                            # per-op overhead and PE<->ACT handoffs)
                            s_ps = ps_s.tile([P, 2 * QW], F32, tag="s")
                            for half in range(2):
                                kc = kc2 + half
                                nc.tensor.matmul(
                                    s_ps[:, half * QW : (half + 1) * QW],
                                    lhsT=kt[
                                        h * DK : (h + 1) * DK, kc * P : (kc + 1) * P
                                    ],
                                    rhs=qt[
                                        h * DK : (h + 1) * DK,
                                        qn * QW : (qn + 1) * QW,
                                    ],
                                    start=True,
                                    stop=True,
                                )
                            e = exp_pool.tile([P, 2 * QW], F32R, tag="exp")
                            nc.scalar.activation(out=e, in_=s_ps, func=Exp, scale=0.125)
                            for half in range(2):
                                j = kc2 + half - 4 * qn
                                if j >= 0:  # diagonal block: causal staircase
                                    nc.vector.tensor_mul(
                                        e[:, half * QW : (half + 1) * QW],
                                        e[:, half * QW : (half + 1) * QW],
                                        mask_sb[:, 384 - 128 * j : 896 - 128 * j],
                                    )
                            es[h] = e
                        for h in range(NH):
                            for half in range(2):
                                kc = kc2 + half
                                nc.tensor.matmul(
                                    ctx_ps[h],
                                    lhsT=v_sb[:, kc, h * 65 : h * 65 + 65],
                                    rhs=es[h][:, half * QW : (half + 1) * QW],
                                    start=(kc == 0),
                                    stop=(kc == nkc - 1),
                                )
                    for h in range(NH):
                        nc.vector.tensor_copy(
                            ctxt[h * DK : (h + 1) * DK, qn * QW : (qn + 1) * QW],
                            ctx_ps[h][0:DK, :],
                        )
                        nc.vector.tensor_copy(
                            den[h * DK : h * DK + 1, qn * QW : (qn + 1) * QW],
                            ctx_ps[h][64:65, :],
                        )

                # ---- batched normalization: 1/den = exp(-ln(den)) on ScalarE
                # over all (h, qn) at once, then broadcast + in-place scale ----
                lnd = rec_pool.tile([65, T], F32, tag="rec")
                rcp = rec_pool.tile([65, T], F32, tag="rcp")
                for h in range(NH):
                    dp = h * DK
                    nc.scalar.activation(
                        out=lnd[dp : dp + 1, :], in_=den[dp : dp + 1, :], func=Log
                    )
                    nc.scalar.activation(
                        out=rcp[dp : dp + 1, :],
                        in_=lnd[dp : dp + 1, :],
                        func=Exp,
                        scale=-1.0,
                    )
                for qn in range(QN):
                    for h in range(NH):
                        bc_ps = ps_proj.tile(
                            [DK, QW], F32, tag="proj", name=f"bc{b}{h}{qn}"
                        )
                        nc.tensor.matmul(
                            bc_ps,
                            lhsT=ones_sb[h * DK : h * DK + 1, :],
                            rhs=rcp[h * DK : h * DK + 1, qn * QW : (qn + 1) * QW],
                            start=True,
                            stop=True,
                        )
                        nc.vector.tensor_mul(
                            ctxt[h * DK : (h + 1) * DK, qn * QW : (qn + 1) * QW],
                            ctxt[h * DK : (h + 1) * DK, qn * QW : (qn + 1) * QW],
                            bc_ps,
                        )

                # ---- output projection (partial over this core's heads) ----
                for rc in range(T // P):
                    for c2 in range(D // QW):
                        ps = ps_proj.tile([P, QW], F32, tag="proj")
                        nc.tensor.matmul(
                            ps,
                            lhsT=ctxt[:, rc * P : (rc + 1) * P],
                            rhs=wo_sb[:, c2 * QW : (c2 + 1) * QW],
                            start=True,
                            stop=True,
                        )
                        ob = ob_pool.tile([P, QW], F32, tag="ob")
                        nc.vector.tensor_copy(ob, ps)
                        nc.sync.dma_start(
                            out=out_d[
                                b * T + rc * P : b * T + (rc + 1) * P,
                                c2 * QW : (c2 + 1) * QW,
                            ],
                            in_=ob,
                        )

    _split_multi_waits(nc)
    _PROGRAM = nc
    return nc


def _make_mask():
    # mask[i, u] = 1.0 if u >= i + 384 else 0.0   (shape [128, 896])
    i = np.arange(P)[:, None]
    u = np.arange(896)[None, :]
    return (u >= i + 384).astype(np.float32)


def make_in_maps(x, Wq, Wk, Wv, Wo):
    import ml_dtypes

    nd = ml_dtypes.bfloat16 if F32R == BF16 else np.float32
    x = np.asarray(x, dtype=np.float32)
    xt = np.ascontiguousarray(x.reshape(B * T, D).T).astype(nd)  # [1024, 4096]
    mask = _make_mask().astype(nd)
    ident = np.eye(P, dtype=np.float32).astype(nd)
    Wq, Wk, Wv, Wo = (np.asarray(w, dtype=np.float32) for w in (Wq, Wk, Wv, Wo))
    in_maps = []
    for c in range(NCORES):
        cols = slice(c * P, (c + 1) * P)
        in_maps.append(
            {
                "xt": xt,
                "wq": np.ascontiguousarray(Wq[:, cols]).astype(nd),
                "wk": np.ascontiguousarray(Wk[:, cols]).astype(nd),
                "wv": np.ascontiguousarray(Wv[:, cols]).astype(nd),
                "wo": np.ascontiguousarray(Wo[cols, :]).astype(nd),
                "mask": mask,
                "ident": ident,
            }
        )
    return in_maps


def kernel(x, Wq, Wk, Wv, Wo):
    from concourse.bass_utils import run_bass_kernel_spmd

    nc = build_program()
    in_maps = make_in_maps(x, Wq, Wk, Wv, Wo)
    res = run_bass_kernel_spmd(nc, in_maps, core_ids=list(range(NCORES)))
    acc = np.zeros((B * T, D), dtype=np.float64)
    for c in range(NCORES):
        acc += res.results[c]["out"]
    return acc.astype(np.float32).reshape(B, T, D)


if __name__ == "__main__":
    rng = np.random.default_rng(0)
    s = 1.0 / np.sqrt(D)
    ins = {
        "x": rng.standard_normal((B, T, D)).astype(np.float32),
        "Wq": (rng.standard_normal((D, D)) * s).astype(np.float32),
        "Wk": (rng.standard_normal((D, D)) * s).astype(np.float32),
        "Wv": (rng.standard_normal((D, D)) * s).astype(np.float32),
        "Wo": (rng.standard_normal((D, D)) * (1.0 / np.sqrt(D))).astype(np.float32),
    }
    out = kernel(**ins)
    print("out", out.shape, out.dtype, float(np.abs(out).max()))
